# revision 14
# baseline (speedup 1.0000x reference)
"""ColorGNN Trainium2 kernel v2: 3-layer message passing on the complete
bipartite graph (50000 birds x 16 colors, H=128), sharded by birds across 8
NeuronCores.

Reformulation (same algebra as v1, now in fp16 end-to-end, rel ~7.6e-4):
    h^0      = relu(p * u0 + A0[i] + B0[c] + c0),   u0 = edge_W @ W1c0
    h^l      = relu(h^{l-1} @ Wf_l + A_l[i] + B_l[c] + c_l), Wf_l = W2_{l-1} @ W1c_l
    aggr     = (sum h) @ W2_l + deg*eb2_l   (aggregation commutes with W2)
  Per-edge adds A[i]+B[c] ride a second accumulating matmul with a constant
  two-hot rhs (bird-major edges, 111 birds + 16 colors = 127 of 128 rows).

v3 collective plan (the decisive fix): the HW AllReduce instruction BLOCKS
its issuing engine queue until the collective completes (doorbell +
wait_ge on the same sequencer), and an 8-core AR has ~100-300us latency.
v2 issued 4 ARs on the Pool queue, which also ran per-tile work ->
each AR stalled Pool mid-pipeline and cascaded into PE stalls (~1.3ms).
v3:
  - ONE AllReduce per layer (2 total): both hacc halves reduce into a
    single [H, 2C] payload; the consumer splits and adds the halves.
  - The Pool queue carries ONLY the collectives and post-collective DMA
    kicks (ab B-row loads): Pool blocking is then harmless.
    * layer-0 colorsum adds: fully on DVE (no Pool split).
    * layers 1-2 bird-sum: no Pool halving; 16 accumulating matmuls ride
      the z psum group on PE (like layer 0), +888 cols/tile on PE.
  - Layer tail order: AR launch first, then every cc-independent prep of
    the next layer, then the cc consumers (csg loads on the DVE queue so
    the SP/DMA queue never waits on the collective).
Engine/memory plan otherwise as v2: fp16 end-to-end, psum-chunk relu
alternating ACT/DVE, h tiles in one in-place SBUF region (>= HRES
round-trip DRAM), prep runs AHEAD tiles early.
"""

import numpy as np
import ml_dtypes

import concourse.bass as bass
import concourse.mybir as mybir
import concourse.tile as tile
from concourse.bass_utils import run_bass_kernel_spmd

F32 = mybir.dt.float32
F16 = mybir.dt.float16
AF = mybir.ActivationFunctionType
ALU = mybir.AluOpType

NCORES = 8
N, C, H, L = 50000, 16, 128, 3
NB = N // NCORES            # 6250 birds per core
TB = 111                    # birds per tile (+1 u0 row +16 B rows = 128)
NT = (NB + TB - 1) // TB    # 57 tiles (last has 34 birds)
NE = TB * C                 # 1776 edge columns per full tile
CHUNKS = [(0, 512), (512, 512), (1024, 512), (1536, 240)]  # psum chunks, 1 bank
HRES = 31                   # h tiles resident in SBUF (in-place all layers)

# relu engine per (layer, chunk-index): one piece per 512-col psum chunk, so
# each piece fires as soon as its own matmul retires and the 4-deep psum ring
# decouples PE from relu latency. Mix balanced per layer phase.
RELU_PLAN = {
    0: ["act", "dve", "act", "dve"],
    1: ["act", "dve", "act", "dve"],
    2: ["act", "dve", "act", "dve"],
}


def _split_multi_waits(nc):
    """walrus in this env allows only ONE sync-wait per instruction. For any
    instruction with more waits, hoist the extras onto same-engine nops
    inserted immediately before it (sequencers execute in program order)."""
    k = 0
    for f in nc.m.functions:
        for blk in f.blocks:
            insts = blk.instructions
            out = []
            for inst in insts:
                si = inst.sync_info
                if si is not None and si.on_wait and len(si.on_wait) > 1:
                    waits = list(si.on_wait)
                    for w in waits[:-1]:
                        nop = mybir.InstNoOp(
                            name=f"waitnop-{k}", engine=inst.engine
                        )
                        k += 1
                        nop.sync_info = mybir.SyncInfo(on_wait=[w], on_update=[])
                        out.append(nop)
                    si.on_wait = waits[-1:]
                out.append(inst)
            if len(out) != len(insts):
                blk.instructions = out


def _f16(a):
    return np.ascontiguousarray(np.asarray(a, np.float64)).astype(np.float16)


def _f32(a):
    return np.ascontiguousarray(np.asarray(a, np.float64).astype(np.float32))


def _consts(inp):
    """Host-side weight folding. Returns dict of name -> np array (replicated)."""
    f = {k: np.asarray(v, np.float64) for k, v in inp.items()}
    eW1, eb1, eW2, eb2 = f["eW1"], f["eb1"], f["eW2"], f["eb2"]
    nW1, nb1, nW2, nb2 = f["nW1"], f["nb1"], f["nW2"], f["nb2"]
    W1a = [eW1[l][:H] for l in range(L)]
    W1b = [eW1[l][H : 2 * H] for l in range(L)]
    W1c = [eW1[l][2 * H :] for l in range(L)]
    W2 = [eW2[l] for l in range(L)]
    U1 = [nW1[l][:H] for l in range(L)]
    U2 = [nW1[l][H:] for l in range(L)]

    c = {}
    c["u0"] = _f16(f["edge_W"][0] @ W1c[0]).reshape(1, H)  # lhsT [1, H]
    cvec = [
        f["edge_b"] @ W1c[0] + eb1[0],
        eb2[0] @ W1c[1] + eb1[1],
        eb2[1] @ W1c[2] + eb1[2],
    ]
    wmats, vvecs = {}, {}
    for l in range(L):
        vvecs[f"cvec{l}"] = cvec[l]
        wmats[f"W1a{l}"] = W1a[l]               # rhs [H, H]
        wmats[f"W1b{l}"] = W1b[l]               # rhs [H, H]
        wmats[f"U1{l}"] = U1[l]                 # lhsT [H, H]
        wmats[f"W2U2{l}"] = W2[l] @ U2[l]       # lhsT [H, H]
        wmats[f"V{l}"] = nW2[l]                 # lhsT [H, H]
        vvecs[f"bb{l}"] = nb1[l] + C * (eb2[l] @ U2[l])
        vvecs[f"nb2{l}"] = nb2[l]
        if l < L - 1:
            vvecs[f"bc{l}"] = nb1[l] + N * (eb2[l] @ U2[l])
    wmats["W2U2cs0"] = (W2[0] @ U2[0]) * 64.0   # lhsT [H, H], csum/64 path
    wmats["W2U2cs1"] = (W2[1] @ U2[1]) * 64.0
    wmats["Wf1"] = W2[0] @ W1c[1]               # lhsT [H, H]
    wmats["Wf2"] = W2[1] @ W1c[2]               # lhsT [H, H]
    vvecs["node_b"] = f["node_b"]
    # layer-0 folds: x0 = p @ node_W + node_b never materialized on device
    vvecs["cvec0"] = vvecs["cvec0"] + f["node_b"] @ W1a[0]
    vvecs["bb0"] = vvecs["bb0"] + f["node_b"] @ U1[0]
    c["NW1a"] = _f16(f["node_W"] @ W1a[0])       # [C, H]
    c["NWU1"] = _f16(f["node_W"] @ U1[0])        # [C, H]
    c["wpack"] = _f16(np.concatenate([wmats[k] for k in sorted(wmats)], axis=1))
    c["vpack"] = _f32(np.stack([vvecs[k] for k in sorted(vvecs)], axis=1))
    c["xc0T"] = _f16((f["node_W"] + f["node_b"]).T)  # [H, C]
    c["color_W"] = _f16(f["color_W"])            # lhsT [H, C]
    c["color_b"] = _f32(f["color_b"]).reshape(C, 1)
    # two-hot base: col e -> 1 at row e//16 (bird) and row TB+1+e%16
    # (color); row TB is reserved for per-tile p data (l=0) / zero (l>=1)
    oh = np.zeros((H, NE), np.float32)
    e = np.arange(NE)
    oh[e // C, e] = 1.0
    oh[TB + 1 + (e % C), e] = 1.0
    c["twohot"] = oh.astype(np.float16)
    return c


def build_nc(sim_mode=False, skew=2, hin_bufs=4, hout_bufs=5,
             hps_bufs=4, sps_bufs=4, split_waits=True, fake_cc=False,
             cc_async=False):
    nc = bass.Bass(num_devices=1 if sim_mode else NCORES)

    wnames = sorted(
        [f"{nm}{l}" for l in range(L) for nm in ("W1a", "W1b", "U1", "W2U2", "V")]
        + ["Wf1", "Wf2", "W2U2cs0", "W2U2cs1"]
    )
    vnames = sorted(
        [f"cvec{l}" for l in range(L)] + [f"bb{l}" for l in range(L)]
        + [f"nb2{l}" for l in range(L)] + ["bc0", "bc1", "node_b"]
    )
    cshapes = {
        "u0": ([1, H], F16),
        "xc0T": ([H, C], F16), "color_W": ([H, C], F16), "color_b": ([C, 1], F32),
        "twohot": ([H, NE], F16),
        "NW1a": ([C, H], F16), "NWU1": ([C, H], F16),
        "wpack": ([H, len(wnames) * H], F16),
        "vpack": ([H, len(vnames)], F32),
    }

    dins = {}
    for name, (shp, dt) in cshapes.items():
        dins[name] = nc.declare_dram_parameter(name, shp, dt, isOutput=False)
    dins["pTf"] = nc.declare_dram_parameter("pTf", [C, NB], F16, isOutput=False)
    dins["pbf"] = nc.declare_dram_parameter("pbf", [1, NB * C], F16, isOutput=False)
    out_d = nc.declare_dram_parameter("outT", [C, NB], F32, isOutput=True)

    with tile.TileContext(nc) as tc:
        with (
            tc.tile_pool(name="const", bufs=1) as constp,
            tc.tile_pool(name="xpool", bufs=1) as xpool,
            tc.tile_pool(name="hsb", bufs=3) as hsbp,
            tc.tile_pool(name="ptile", bufs=2) as ptp,
            tc.tile_pool(name="small", bufs=4) as smallp,
            tc.tile_pool(name="hps", bufs=hps_bufs, space="PSUM") as hps,
            tc.tile_pool(name="sps", bufs=1, space="PSUM") as sps,
            tc.tile_pool(name="dram", bufs=1, space="DRAM") as dramp,
        ):
            # ---- load constants ----
            cs = {}
            for name, (shp, dt) in cshapes.items():
                cs[name] = constp.tile(shp, dt, name=f"c_{name}")
                nc.sync.dma_start(out=cs[name][:], in_=dins[name][:])
            for i, nm in enumerate(wnames):
                cs[nm] = cs["wpack"][:, i * H : (i + 1) * H]
            for i, nm in enumerate(vnames):
                cs[nm] = cs["vpack"][:, i : i + 1]
            # p data resident in SBUF
            pTf_sb = constp.tile([C, NB], F16, name="pTf_sb")
            nc.sync.dma_start(out=pTf_sb[:], in_=dins["pTf"][:])

            xT = xpool.tile([H, NT * TB], F16)       # bird states, H-major
            xcT = xpool.tile([H, C], F16)            # color states
            nc.sync.dma_start(out=xcT[:], in_=dins["xc0T"][:])

            # h region: resident tiles reused in place across all 3 layers
            hreg = constp.tile([H, HRES * NE], F16, name="hreg")
            h_d = dramp.tile([H, NT * NE], F16, name="h_d")  # overflow tiles
            # colorsum accumulators (fp16, 2x DVE adds), tiles split by parity
            hacc = [xpool.tile([H, NE], F16, name=f"hacc{i}") for i in range(2)]

            AB_SLOTS = 4
            ab_tiles = [xpool.tile([H, H], F16, name=f"abslot{i}")
                        for i in range(AB_SLOTS)]
            for abt in ab_tiles:   # row TB = u0; only read in pass 0
                nc.sync.dma_start(out=abt[TB : TB + 1, :], in_=dins["u0"][:])
            P3H_SLOTS = 3
            p3h = [xpool.tile([H, NE], F16, name=f"p3h{i}")
                   for i in range(P3H_SLOTS)]
            for ph3 in p3h:        # three-hot base; row TB overwritten per tile
                nc.sync.dma_start(out=ph3[:], in_=dins["twohot"][:])
            cc_in = [dramp.tile([H, 2 * C], F32, name=f"cc_in{i}") for i in range(2)]
            cc_out = [dramp.tile([H, 2 * C], F32, name=f"cc_out{i}") for i in range(2)]

            def tcols(t):
                nb = min(TB, NB - t * TB)
                return t * TB, nb

            # Tile processing order: interleave resident and DRAM-round-trip
            # tiles so the h DMA traffic spreads across the whole layer
            # instead of saturating the DMA engines in one burst.
            res, nres = list(range(HRES)), list(range(HRES, NT))
            TORDER = []
            ri = ni = 0
            for k in range(NT):
                take_res = ni * len(res) >= ri * len(nres) if nres else True
                if ri < len(res) and (ni >= len(nres) or take_res):
                    TORDER.append(res[ri]); ri += 1
                else:
                    TORDER.append(nres[ni]); ni += 1
            POS = {t: i for i, t in enumerate(TORDER)}

            def h_home(t):
                """SBUF home of h tile t (resident region) or None."""
                if t < HRES:
                    return hreg[:, t * NE : (t + 1) * NE]
                return None

            prepped = {}

            def stage_prep(lp, t):
                """Prefetch for layer lp, tile t: p-row / h-in DMA, A matmul,
                ab copy. Runs AHEAD tiles before the h matmuls consume them
                (and across the layer boundary, during the tail collective)."""
                t0, nb = tcols(t)
                ne = nb * C
                ec0 = t * NE
                rh = hin = None
                if lp == 0:
                    rh = p3h[POS[t] % P3H_SLOTS]
                    nc.sync.dma_start(out=rh[TB : TB + 1, :ne],
                                      in_=dins["pbf"][:, t0 * C : t0 * C + ne])
                else:
                    hin = h_home(t)
                    if hin is None:
                        hin = hsbp.tile([H, NE], F16, tag="hin", name="hin",
                                        bufs=hin_bufs)
                        nc.sync.dma_start(out=hin[:, :ne],
                                          in_=h_d[:, ec0 : ec0 + ne])
                # A_l = x^l @ W1a_l, bird-major [nb, H]
                a_ps = sps.tile([TB, H], F32, name="smp", tag="aps", bufs=1)
                if lp == 0:
                    nc.tensor.matmul(a_ps[:nb, :],
                                     lhsT=pTf_sb[:, t0 : t0 + nb],
                                     rhs=cs["NW1a"][:], start=True, stop=True)
                else:
                    nc.tensor.matmul(a_ps[:nb, :], lhsT=xT[:, t0 : t0 + nb],
                                     rhs=cs[f"W1a{lp}"][:], start=True, stop=True)
                ab = ab_tiles[POS[t] % AB_SLOTS]
                if lp == 1:
                    nc.scalar.copy(ab[:nb, :], a_ps[:nb, :])
                else:
                    nc.vector.tensor_copy(ab[:nb, :], a_ps[:nb, :])
                prepped[(lp, t)] = (rh, hin, ab)

            # ================= pass l =================
            for l in range(L):
                last = l == L - 1
                # B_l [C, H] for the two-hot lhsT rows 112:128
                src = cs["xc0T"] if l == 0 else xcT
                b_ps = sps.tile([C, H], F32, name="smp", tag="sm")
                nc.tensor.matmul(b_ps[:], lhsT=src[:], rhs=cs[f"W1b{l}"][:],
                                 start=True, stop=True)
                B_sb = smallp.tile([C, H], F16)
                nc.scalar.copy(B_sb[:], b_ps[:])
                for abt in ab_tiles:
                    # Pool queue: idle except collectives; for l>=1 these are
                    # cc-dependent anyway (B_sb <- xcT <- AllReduce)
                    nc.gpsimd.dma_start(out=abt[TB + 1 :, :], in_=B_sb[:])
                if l == 0:
                    nc.vector.memset(hacc[0][:], 0.0)
                    nc.vector.memset(hacc[1][:], 0.0)

                def stage_h(t):
                    """h matmuls + relu, store h, aggregates."""
                    t0, nb = tcols(t)
                    ne = nb * C
                    ec0 = t * NE
                    rh, hin, ab = prepped.pop((l, t))

                    # ---- h tile: psum halves + relu ----
                    home = h_home(t)
                    if home is not None:
                        h_sb = home   # in-place: overwrites h^{l-1}[t]
                    else:
                        h_sb = hsbp.tile([H, NE], F16, tag="hout", name="h_sb",
                                         bufs=hout_bufs)
                    for ci, (cbase, cwidth) in enumerate(CHUNKS):
                        cw = min(cwidth, ne - cbase)
                        if cw <= 0:
                            continue
                        sl = slice(cbase, cbase + cw)
                        ps = hps.tile([H, 512], F32, name="hps")
                        if l == 0:
                            nc.tensor.matmul(ps[:, :cw], lhsT=ab[:],
                                             rhs=rh[:, sl],
                                             start=True, stop=True)
                        else:
                            nc.tensor.matmul(ps[:, :cw],
                                             lhsT=cs[f"Wf{l}"][:],
                                             rhs=hin[:, sl],
                                             start=True, stop=False)
                            nc.tensor.matmul(ps[:, :cw], lhsT=ab[:],
                                             rhs=cs["twohot"][:, sl],
                                             start=False, stop=True)
                        eng = RELU_PLAN[l][ci]
                        dst = h_sb[:, sl]
                        src = ps[:, :cw]
                        if eng == "act":
                            nc.scalar.activation(dst, src, AF.Relu,
                                                 bias=cs[f"cvec{l}"][:])
                        elif eng == "pool":
                            nc.vector.tensor_scalar(
                                dst, src, cs[f"cvec{l}"][:], 0.0,
                                op0=ALU.add, op1=ALU.max)
                        else:
                            nc.vector.tensor_scalar(
                                dst, src, cs[f"cvec{l}"][:], 0.0,
                                op0=ALU.add, op1=ALU.max)

                    return h_sb

                def stage_node(t, h_sb):
                    """h-tile consumers + bird node update for tile t. Runs
                    `skew` tiles after stage_h(t) so every wait on the split
                    relu is already satisfied when it reaches a sequencer
                    (multi-wait nops block the SEQ until their sem fires)."""
                    t0, nb = tcols(t)
                    ne = nb * C
                    ec0 = t * NE
                    if not last:
                        # colorsum accumulate fully on DVE (Pool must stay
                        # empty for the collectives); parity split gives two
                        # independent accumulation chains
                        acc = hacc[POS[t] % 2]
                        nc.vector.tensor_add(acc[:, :ne], acc[:, :ne],
                                             h_sb[:, :ne])
                    # bird-sum rides the z psum group on PE for ALL layers:
                    # 16 strided accumulating matmuls (no Pool halving)
                    z_ps = sps.tile([H, TB], F32, name="smp", tag="zps", bufs=1)
                    if l == 0:
                        nc.tensor.matmul(z_ps[:, :nb], lhsT=cs["NWU1"][:],
                                         rhs=pTf_sb[:, t0 : t0 + nb],
                                         start=True, stop=False)
                    else:
                        nc.tensor.matmul(z_ps[:, :nb], lhsT=cs[f"U1{l}"][:],
                                         rhs=xT[:, t0 : t0 + nb],
                                         start=True, stop=False)
                    h3 = h_sb[:, : nb * C].rearrange("p (b c) -> p b c", c=C)
                    for cc in range(C):
                        nc.tensor.matmul(z_ps[:, :nb], lhsT=cs[f"W2U2{l}"][:],
                                         rhs=h3[:, :, cc], start=False,
                                         stop=(cc == C - 1))
                    if not last and h_home(t) is None:
                        nc.sync.dma_start(out=h_d[:, ec0 : ec0 + ne],
                                          in_=h_sb[:, :ne])
                    s_sb = smallp.tile([H, TB], F16, tag="ssb", name="s_sb")
                    nc.scalar.activation(s_sb[:, :nb], z_ps[:, :nb], AF.Relu,
                                         bias=cs[f"bb{l}"][:])
                    x_ps = sps.tile([H, TB], F32, name="smp", tag="xps", bufs=1)
                    nc.tensor.matmul(x_ps[:, :nb], lhsT=cs[f"V{l}"][:],
                                     rhs=s_sb[:, :nb], start=True, stop=True)
                    nc.scalar.activation(xT[:, t0 : t0 + nb], x_ps[:, :nb],
                                         AF.Identity, bias=cs[f"nb2{l}"][:])

                    if last:
                        # scores_T = color_W.T @ x3 + color_b ; out = scores * p
                        sc_ps = sps.tile([C, TB], F32, name="smp", tag="sm")
                        nc.tensor.matmul(sc_ps[:, :nb], lhsT=cs["color_W"][:],
                                         rhs=xT[:, t0 : t0 + nb],
                                         start=True, stop=True)
                        sc_sb = smallp.tile([C, TB], F32, tag="scsb", name="sc_sb")
                        nc.scalar.activation(sc_sb[:, :nb], sc_ps[:, :nb],
                                             AF.Identity, bias=cs["color_b"][:])
                        o_sb = smallp.tile([C, TB], F32, tag="osb", name="o_sb")
                        nc.vector.tensor_mul(o_sb[:, :nb], sc_sb[:, :nb],
                                             pTf_sb[:, t0 : t0 + nb])
                        nc.sync.dma_start(out=out_d[:, t0 : t0 + nb],
                                          in_=o_sb[:, :nb])

                def launch_cc():
                    """Reduce both hacc halves into one [H, 2C] payload and
                    launch the layer's single AllReduce on the (otherwise
                    empty) Pool queue. The consumer adds the two halves."""
                    for part in range(2):
                        h3a = hacc[part][:].rearrange("p (b c) -> p b c", c=C)
                        csum = smallp.tile([H, C], F32, tag=f"csum{part}")
                        nc.vector.tensor_reduce(csum[:],
                                                h3a.transpose([0, 2, 1]),
                                                axis=mybir.AxisListType.X,
                                                op=ALU.add)
                        nc.sync.dma_start(
                            out=cc_in[l][:, part * C : (part + 1) * C],
                            in_=csum[:])
                        if l < L - 2:
                            nc.vector.memset(hacc[part][:], 0.0)
                    if sim_mode or fake_cc:
                        nc.sync.dma_start(out=cc_out[l][:], in_=cc_in[l][:])
                    else:
                        nc.gpsimd.collective_compute(
                            "AllReduce", ALU.add,
                            replica_groups=[list(range(NCORES))],
                            ins=[cc_in[l][:].opt()],
                            outs=[cc_out[l][:].opt()],
                        )

                # software pipeline: prep runs AHEAD tiles early, node update
                # lags by `skew` tiles, so PE/ACT never stall on copies/DMA
                AHEAD = 3
                pend = {}
                for st in range(-AHEAD, NT + skew):
                    if 0 <= st + AHEAD < NT:
                        tt = TORDER[st + AHEAD]
                        if (l, tt) not in prepped:
                            stage_prep(l, tt)
                    if 0 <= st < NT:
                        pend[st] = stage_h(TORDER[st])
                    if 0 <= st - skew:
                        stage_node(TORDER[st - skew], pend.pop(st - skew))

                # ---- layer tail: color update (l < 2) ----
                if not last:
                    # AllReduce launches first; then every cc-independent
                    # prep of the next layer; only then the cc consumers
                    launch_cc()
                    for tt in TORDER[: AHEAD + 3]:
                        stage_prep(l + 1, tt)
                    # csg load on the ACT queue: the SP/DMA queue must never
                    # wait on the collective (it feeds all tile DMA traffic),
                    # and everything later on ACT is cc-downstream anyway
                    csg = smallp.tile([H, 2 * C], F32, tag="csg")
                    csg_src = cc_in[l] if cc_async else cc_out[l]
                    nc.scalar.dma_start(out=csg[:], in_=csg_src[:])
                    csg_t = smallp.tile([H, C], F32, tag="csgt")
                    nc.vector.tensor_add(csg_t[:], csg[:, :C], csg[:, C:])
                    csg_f = smallp.tile([H, C], F16, tag="csgf")
                    nc.vector.tensor_scalar(csg_f[:], csg_t[:], 1.0 / 64.0,
                                            None, op0=ALU.mult)
                    zc_ps = sps.tile([H, C], F32, name="smp", tag="sm")
                    nc.tensor.matmul(zc_ps[:], lhsT=cs[f"U1{l}"][:], rhs=xcT[:],
                                     start=True, stop=False)
                    nc.tensor.matmul(zc_ps[:], lhsT=cs[f"W2U2cs{l}"][:],
                                     rhs=csg_f[:], start=False, stop=True)
                    sc2 = smallp.tile([H, C], F16, tag="sc2")
                    nc.scalar.activation(sc2[:], zc_ps[:], AF.Relu,
                                         bias=cs[f"bc{l}"][:])
                    xc_ps = sps.tile([H, C], F32, name="smp", tag="sm")
                    nc.tensor.matmul(xc_ps[:], lhsT=cs[f"V{l}"][:], rhs=sc2[:],
                                     start=True, stop=True)
                    nc.scalar.activation(xcT[:], xc_ps[:], AF.Identity,
                                         bias=cs[f"nb2{l}"][:])

    if split_waits:
        _split_multi_waits(nc)
    return nc


def make_in_maps(inputs):
    c = _consts(inputs)
    probs = np.asarray(inputs["probs"], np.float32)
    in_maps = []
    for k in range(NCORES):
        sl = probs[k * NB : (k + 1) * NB]          # [NB, C]
        m = dict(c)
        m["pTf"] = np.ascontiguousarray(sl.T).astype(np.float16)   # [C, NB]
        m["pbf"] = sl.reshape(1, -1).astype(np.float16)            # [1, NB*C]
        in_maps.append(m)
    return in_maps


_NC_CACHE = None


def kernel(**inputs) -> np.ndarray:
    global _NC_CACHE
    if _NC_CACHE is None:
        _NC_CACHE = build_nc()
    nc = _NC_CACHE
    in_maps = make_in_maps(inputs)
    res = run_bass_kernel_spmd(nc, in_maps, core_ids=list(range(NCORES)))
    outT = np.concatenate([res.results[k]["outT"] for k in range(NCORES)], axis=1)
    return np.ascontiguousarray(outT.T).astype(np.float32)

